# revision 25
# baseline (speedup 1.0000x reference)
"""Top-8-per-row kernel for x[2048, 32768] fp32 on 8 TRN2 NeuronCores.

Strategy: data-parallel over rows (256 rows/core = 2 partition blocks of
128). Stream column tiles into SBUF, use the DVE InstMax (top-8 per
partition, descending) per tile, then a final InstMax over the per-tile
candidates, then reverse to ascending order and DMA out.
"""

from contextlib import ExitStack

import numpy as np

import concourse.bass as bass
import concourse.tile as tile
from concourse import bacc, mybir
from concourse.bass_utils import run_bass_kernel_spmd

B = 2048
N = 32768
K = 8
N_CORES = 8
ROWS_PER_CORE = B // N_CORES  # 256
P = 128
N_BLOCKS = ROWS_PER_CORE // P  # 2
# Column tile sizes per 128-row block (max8 granularity = DMA
# granularity). 4096 cols -> 16KB partition lines, which keep all 16
# SDMA engines at line rate (~421 GB/s aggregate measured); 32KB lines
# trip a slow path on one engine. Interleaved A/B benching showed
# uniform 4096 beats both 8192-based supertiles and tapered tails.
TAPER = [4096] * 8
DMA_C = 4096
DATA_BUFS = 4
F32 = mybir.dt.float32
assert sum(TAPER) == N


def _build(
    taper=None,
    data_bufs: int = DATA_BUFS,
    dma_c: int = DMA_C,
    dma_reverse: bool = False,
    head_gpsimd: int = 0,
    last_taper=None,
    store_eng: str = "sync",
) -> bass.Bass:
    taper = list(TAPER if taper is None else taper)
    tapers = [taper] * (N_BLOCKS - 1) + [list(last_taper or taper)]
    assert all(sum(tp) == N for tp in tapers)
    nc = bacc.Bacc(
        "TRN2", target_bir_lowering=False, debug=False, num_devices=N_CORES
    )
    x = nc.dram_tensor("x", [ROWS_PER_CORE, N], F32, kind="ExternalInput").ap()
    out = nc.dram_tensor("out", [ROWS_PER_CORE, K], F32, kind="ExternalOutput").ap()

    with ExitStack() as ctx:
        tc = ctx.enter_context(tile.TileContext(nc))
        data_pool = ctx.enter_context(tc.tile_pool(name="data", bufs=data_bufs))
        small_pool = ctx.enter_context(tc.tile_pool(name="small", bufs=2 * N_BLOCKS))

        for b in range(N_BLOCKS):
            rows = slice(b * P, (b + 1) * P)
            tp = tapers[b]
            n_tiles = len(tp)
            offs = [sum(tp[:i]) for i in range(n_tiles)]
            cands = small_pool.tile([P, K * n_tiles], F32, tag="cands")
            for t, (off, sz) in enumerate(zip(offs, tp)):
                d = data_pool.tile([P, sz], F32, tag="data")
                # First loads go through GPSIMD's SWDGE ring: it clears
                # its preamble before the Sync sequencer does, so the
                # stream starts earlier.
                eng = nc.gpsimd if (b == 0 and t < head_gpsimd) else nc.sync
                if b == N_BLOCKS - 1 and sz < dma_c and t == n_tiles - 1:
                    # Final sub-size tile rides the Scalar HWDGE ring so
                    # it streams concurrently with the Sync ring's
                    # previous chunk instead of serializing behind it.
                    eng = nc.scalar
                for c0 in range(0, sz, dma_c):
                    c1 = min(c0 + dma_c, sz)
                    eng.dma_start(d[:, c0:c1], x[rows, off + c0 : off + c1])
                nc.vector.max(cands[:, t * K : (t + 1) * K], d[:])
            top = small_pool.tile([P, K], F32, tag="top")
            nc.vector.max(top[:], cands[:])
            # Stores ride a non-Sync queue by default: a store on the Sync
            # ring makes the sequencer block on the reduction semaphore,
            # stalling issue of the next block's loads (~10us observed).
            seng = getattr(nc, store_eng)
            if dma_reverse:
                # Reverse to ascending on the DMA's SBUF-read side (8
                # elements/partition, descriptor cost is negligible).
                seng.dma_start(out[rows, :], top[:, ::-1])
            else:
                asc = small_pool.tile([P, K], F32, tag="asc")
                nc.vector.tensor_copy(asc[:], top[:, ::-1])
                seng.dma_start(out[rows, :], asc[:])

    nc.compile()
    return nc


# Geometric tail for the final 128-row block: vector consumes a tile in
# 1.081 ns/col but DMA delivers one in 1.204 ns/col, so as long as
# consecutive tiles shrink by no more than ~0.9x the DVE stays
# arrival-gated and the post-last-byte drain is ~the last tile's max8
# (~0.8us) instead of a full 4096 max8 + queued tail (~5us observed).
# Sums to exactly 32768.
GEO_TAIL = [
    3584, 3264, 2944, 2688, 2432, 2176, 1984, 1792, 1600, 1472,
    1344, 1216, 1088, 1024, 960, 896, 832, 768, 704,
]
assert sum(GEO_TAIL) == N
# Deeper taper: ends at 512 cols (drain ~0.55us max8). The 3776 after
# 3584 is fine — only shrink-ratio on decreases matters for the drain.
GEO_TAIL2 = [
    3584, 3776, 3392, 3008, 2688, 2368, 2112, 1856, 1664, 1472,
    1280, 1152, 1024, 896, 768, 640, 576, 512,
]
assert sum(GEO_TAIL2) == N
GEO_TAIL3 = [
    3648, 3264, 2944, 2624, 2368, 2112, 1920, 1728, 1536, 1408, 1280,
    1152, 1024, 896, 832, 768, 704, 640, 576, 512, 448, 384,
]
assert sum(GEO_TAIL3) == N


def _build_raw(
    taper=None,
    data_bufs: int = DATA_BUFS,
    dma_c: int = DMA_C,
    last_taper=None,
    dma_reverse: bool = False,
    pre_reduce: bool = True,
    head_split: bool = False,
    no_gpsimd_drain: bool = False,
    gpsimd_head: bool = False,
    store_mode: str = "scalar",
    pre_depth: int = 1,
) -> bass.Bass:
    """Manual-semaphore variant: no TileContext, so none of its
    EVSEM-butterfly preamble/exit barriers. Sync issues loads, Vector
    does the max8 chain, Scalar issues stores and holds the final
    completion wait.

    last_taper: column tiling for the final 128-row block. A small final
    tile keeps the last serial max8 off the critical path. pre_reduce:
    reduce all-but-last candidate groups while the last tile streams, so
    only a 16-wide max8 remains after the final tile's max8."""
    taper = list(TAPER if taper is None else taper)
    last_taper = list(taper if last_taper is None else last_taper)
    tapers = [list(taper)] * (N_BLOCKS - 1) + [last_taper]
    assert all(sum(tp) == N for tp in tapers)
    assert all(sz <= dma_c or sz % dma_c == 0 for tp in tapers for sz in tp)
    super_c = max(max(tp) for tp in tapers)
    nc = bacc.Bacc(
        "TRN2", target_bir_lowering=False, debug=False, num_devices=N_CORES
    )
    x = nc.dram_tensor("x", [ROWS_PER_CORE, N], F32, kind="ExternalInput").ap()
    out = nc.dram_tensor("out", [ROWS_PER_CORE, K], F32, kind="ExternalOutput").ap()

    # (block, tile_idx, col_off, cols, n_chunks) in stream order
    tiles_flat = []
    for b in range(N_BLOCKS):
        tp = tapers[b]
        offs = [sum(tp[:i]) for i in range(len(tp))]
        for t, (off, sz) in enumerate(zip(offs, tp)):
            nch = (sz + dma_c - 1) // dma_c
            tiles_flat.append((b, t, off, sz, nch))
    # Cumulative tile count through block b (vd threshold at block end).
    cum_tiles = []
    acc = 0
    for b in range(N_BLOCKS):
        acc += len(tapers[b])
        cum_tiles.append(acc)
    # Per-buffer-slot load semaphores: a single counting sem across all
    # tiles would be racy (concurrent chunk DMAs from different tiles
    # can mix to hit a threshold), but per-slot counts only saturate
    # when every chunk of that slot's latest tile has landed, because
    # the next tile on the slot isn't issued until the current one is
    # consumed (vd gate).
    slot_thresh = [0] * data_bufs
    tile_thresh = []
    for i, tf in enumerate(tiles_flat):
        s = i % data_bufs
        slot_thresh[s] += 16 * tf[4]
        tile_thresh.append(slot_thresh[s])

    # Engine that issues each load. The Scalar/GpSimd sequencers clear the
    # framework preamble ~1us before Sync (which runs an extra DRAIN), so
    # the first tiles ride their rings to start the stream early. Only
    # SP/Activation have HWDGE; GpSimd DMAs via SWDGE.
    def load_issuer(i):
        if not head_split:
            return "sync"
        if i == 0:
            return "scalar"
        if i == 1 and gpsimd_head:
            return "gpsimd"
        return "sync"

    with ExitStack() as ctx:
        block = ctx.enter_context(nc.Block(no_gpsimd_drain=no_gpsimd_drain))
        ld = [
            ctx.enter_context(nc.semaphore(f"ld{s}")) for s in range(data_bufs)
        ]
        vd = ctx.enter_context(nc.semaphore("vd"))
        pv = ctx.enter_context(nc.semaphore("pv"))
        fin = ctx.enter_context(nc.semaphore("fin"))
        res = ctx.enter_context(nc.semaphore("res"))
        st = ctx.enter_context(nc.semaphore("st"))
        data = [
            ctx.enter_context(nc.sbuf_tensor(f"data{i}", [P, super_c], F32))
            for i in range(data_bufs)
        ]
        cands = [
            ctx.enter_context(
                nc.sbuf_tensor(f"cands{b}", [P, K * len(tapers[b])], F32)
            )
            for b in range(N_BLOCKS)
        ]
        # pcat[:, :K] = pre-reduced top of all early groups; the last
        # pre_depth tiles' max8s write the remaining slots directly.
        pcat = [
            ctx.enter_context(
                nc.sbuf_tensor(f"pcat{b}", [P, (pre_depth + 1) * K], F32)
            )
            for b in range(N_BLOCKS)
        ]
        top = [
            ctx.enter_context(nc.sbuf_tensor(f"top{b}", [P, K], F32))
            for b in range(N_BLOCKS)
        ]
        asc = [
            ctx.enter_context(nc.sbuf_tensor(f"asc{b}", [P, K], F32))
            for b in range(N_BLOCKS)
        ]

        def issue_load(eng, i):
            b, t, off, sz, nch = tiles_flat[i]
            buf = data[i % data_bufs]
            rows = slice(b * P, (b + 1) * P)
            for c0 in range(0, sz, dma_c):
                c1 = min(c0 + dma_c, sz)
                eng.dma_start(
                    out=buf[:, c0:c1], in_=x[rows, off + c0 : off + c1]
                ).then_inc(ld[i % data_bufs], 16)

        def issue_store(eng, b):
            rows = slice(b * P, (b + 1) * P)
            if dma_reverse:
                # Reverse to ascending on the DMA's SBUF-read side.
                eng.wait_ge(fin, b + 1)
                src = top[b][:, ::-1]
            else:
                eng.wait_ge(res, b + 1)
                src = asc[b][:]
            eng.dma_start(out=out[rows, :], in_=src).then_inc(st, 16)

        @block.sync
        def _(sync: bass.BassEngine):
            for i in range(len(tiles_flat)):
                if load_issuer(i) != "sync":
                    continue
                if i >= data_bufs:
                    sync.wait_ge(vd, i - data_bufs + 1)
                issue_load(sync, i)
            if store_mode == "sync_tail":
                # Stores after all load issues: no mid-stream issue stall,
                # no store descriptors cluttering the saturated window, and
                # SP's DGE delay (650ns) beats Activation's (784ns).
                for b in range(N_BLOCKS):
                    issue_store(sync, b)
                sync.wait_ge(st, 16 * N_BLOCKS)

        @block.vector
        def _(vec: bass.BassVectorEngine):
            for i, (b, t, off, sz, nch) in enumerate(tiles_flat):
                buf = data[i % data_bufs]
                n_tiles = len(tapers[b])
                d = (
                    min(pre_depth, n_tiles - 1)
                    if pre_reduce and n_tiles > 2
                    else 0
                )
                if d and t == n_tiles - d:
                    # Fold all earlier groups while the last d tiles are
                    # still streaming (vd covers our own drained writes).
                    vec.wait_ge(vd, cum_tiles[b] - d)
                    vec.max(
                        pcat[b][:, :K], cands[b][:, : K * (n_tiles - d)]
                    ).then_inc(pv, 1)
                vec.wait_ge(ld[i % data_bufs], tile_thresh[i])
                if d and t >= n_tiles - d:
                    j = t - (n_tiles - d)
                    dst = pcat[b][:, (j + 1) * K : (j + 2) * K]
                else:
                    dst = cands[b][:, t * K : (t + 1) * K]
                vec.max(dst, buf[:, :sz]).then_inc(vd, 1)
                if t == n_tiles - 1:
                    # DVE writes drain asynchronously: same-engine RAW
                    # needs a sem wait for visibility, not just program
                    # order.
                    vec.wait_ge(vd, cum_tiles[b])
                    if d:
                        vec.wait_ge(pv, sum(
                            1 for bb in range(b + 1) if len(tapers[bb]) > 2
                        ))
                        vec.max(
                            top[b][:], pcat[b][:, : (d + 1) * K]
                        ).then_inc(fin, 1)
                    else:
                        vec.max(top[b][:], cands[b][:]).then_inc(fin, 1)
                    if not dma_reverse:
                        vec.wait_ge(fin, b + 1)
                        vec.tensor_copy(asc[b][:], top[b][:, ::-1]).then_inc(
                            res, 1
                        )
        if store_mode == "scalar" or head_split:

            @block.scalar
            def _(sc: bass.BassEngine):
                if head_split:
                    issue_load(sc, 0)
                if store_mode == "scalar":
                    for b in range(N_BLOCKS):
                        issue_store(sc, b)
                    sc.wait_ge(st, 16 * N_BLOCKS)

        if head_split and gpsimd_head:

            @block.gpsimd
            def _(gp: bass.BassEngine):
                issue_load(gp, 1)

    nc.compile()
    return nc


def _build_best() -> bass.Bass:
    """Best measured config: raw manual-semaphore kernel, geometric tail
    taper to 512 cols, stores on the Sync ring after all load issues,
    no GpSimd DGE drain at exit, 2-deep candidate pre-reduce.
    ~94.2us vs 97.6us TileContext baseline."""
    return _build_raw(
        last_taper=GEO_TAIL2,
        data_bufs=6,
        store_mode="sync_tail",
        no_gpsimd_drain=True,
        pre_depth=2,
    )


def kernel(x: np.ndarray, k) -> np.ndarray:
    k = int(np.asarray(k))
    assert k == K, f"kernel hardcoded for k={K}, got {k}"
    assert x.shape == (B, N), x.shape
    x = np.ascontiguousarray(x, dtype=np.float32)

    nc = _build_best()
    in_maps = [
        {"x": x[c * ROWS_PER_CORE : (c + 1) * ROWS_PER_CORE]} for c in range(N_CORES)
    ]
    res = run_bass_kernel_spmd(nc, in_maps, list(range(N_CORES)))
    out = np.concatenate([res.results[c]["out"] for c in range(N_CORES)], axis=0)
    return np.asarray(out, dtype=np.float32)


if __name__ == "__main__":
    rng = np.random.default_rng(0)
    xs = rng.standard_normal((B, N), dtype=np.float32)
    got = kernel(xs, 8)
    want = np.sort(xs, axis=1)[:, -K:]
    err = np.max(np.abs(got - want))
    print("absmax err:", err)

